# revision 1
# baseline (speedup 1.0000x reference)
"""Single-head causal self-attention (B=4, T=4096, C=1024, HS=64) on 8 TRN2 cores.

Sharding: core = 2*b + h; the two cores of batch b split the 8 query blocks
(512 rows each) in a load-balanced interleave: h=0 -> blocks {0,3,4,7},
h=1 -> blocks {1,2,5,6} (equal causal-score work: 80 context chunks each).

The SPMD program is identical on every core; per-core differences are pure
data:
  xt  = x[b].T (shared context, global order)
  xtq = x[b, blocks].T (the core's query rows, gathered host-side)
  thr = causal-mask threshold columns (position-aware, per core)
Slot j processes query block g_j against context prefix [0, 128*NCH[j]);
the last 8 context chunks of each slot are masked with data-driven
thresholds against a ramp constant (handles the diagonal, "future" rows
inside the uniform prefix, and fully-masked padding chunks alike).

Dataflow per core (matmul operands bf16, PSUM f32):
  A1: [K^T|V^T] tiles = ([Wk | Wv]).T @ xt      (N=1024 moving, 8 c-chunks)
      V^T -> PE-transpose -> V natural, ones column appended (softmax sums)
  A2: Q^T = (Wq/8).T @ xtq
  C:  S^T piece = K^T_chunk.T @ Q^T_piece        (K-dim = 64, N<=1024)
      E = exp(S^T) (ScalarE, psum->sbuf bf16), mask via precomputed tiles
  D:  O^T[65, q] += [V|1]_chunk.T @ E_piece      (row 64 = softmax sums)
  E:  PE-transpose O^T -> O, out = O[:, :64] * (1 / O[:, 64])
"""

import numpy as np
import ml_dtypes

B, T, C, HS = 4, 4096, 1024, 64
QH = T // 2            # queries per core
NSLOT = 4
NCH = [8, 16, 24, 32]  # uniform context chunks (of 128) per slot
BLOCKS = [[0, 3, 4, 7], [1, 2, 5, 6]]  # global 512-blocks per half
CCH = C // 128

_compiled = None


def _build_program():
    import concourse.bass as bass
    import concourse.mybir as mybir
    import concourse.tile as tile
    from concourse import bacc
    from concourse.masks import make_identity
    from contextlib import ExitStack

    f32 = mybir.dt.float32
    bf16 = mybir.dt.bfloat16

    nc = bacc.Bacc("TRN2", target_bir_lowering=False, debug=False, num_devices=8)

    xt_d = nc.dram_tensor("xt", [C, T], bf16, kind="ExternalInput").ap()
    xtq_d = nc.dram_tensor("xtq", [C, QH], bf16, kind="ExternalInput").ap()
    wkv_d = nc.dram_tensor("wkv", [C, 128], bf16, kind="ExternalInput").ap()
    wq_d = nc.dram_tensor("wq", [C, HS], bf16, kind="ExternalInput").ap()
    ramp_d = nc.dram_tensor("ramp", [128, 512], f32, kind="ExternalInput").ap()
    thr_d = nc.dram_tensor("thr", [128, 32], f32, kind="ExternalInput").ap()
    out_d = nc.dram_tensor("out", [QH, HS], f32, kind="ExternalOutput").ap()

    with tile.TileContext(nc) as tc, ExitStack() as ctx:
        consts = ctx.enter_context(tc.tile_pool(name="consts", bufs=1))
        epool = ctx.enter_context(tc.tile_pool(name="epool", bufs=6))
        mpool = ctx.enter_context(tc.tile_pool(name="mpool", bufs=2))
        opool = ctx.enter_context(tc.tile_pool(name="opool", bufs=4))

        xt = consts.tile([128, CCH, T], bf16)
        xtq = consts.tile([128, CCH, QH], bf16)
        wkv = consts.tile([128, CCH, 128], bf16)
        wq = consts.tile([128, CCH, HS], bf16)
        kT = consts.tile([64, T], bf16)
        qT = consts.tile([64, QH], bf16)
        vp = consts.tile([128, T // 128, HS + 1], bf16)  # [V | ones]
        ramp = consts.tile([128, 512], f32)
        thr = consts.tile([128, 32], f32)
        id_bf = consts.tile([64, 64], bf16)
        id_f32 = consts.tile([65, 65], f32)

        nc.sync.dma_start(out=wkv, in_=wkv_d.rearrange("(a p) m -> p a m", p=128))
        nc.sync.dma_start(out=wq, in_=wq_d.rearrange("(a p) m -> p a m", p=128))
        nc.sync.dma_start(out=ramp, in_=ramp_d)
        nc.sync.dma_start(out=thr, in_=thr_d)
        make_identity(nc, id_bf)
        make_identity(nc, id_f32)
        nc.vector.memset(vp[:, :, HS], 1.0)

        # xtq first (A2 unblocks early), then xt; split across HWDGE/SWDGE
        xtq_r = xtq_d.rearrange("(a p) t -> p a t", p=128)
        for tb in range(QH // 512):
            sl = slice(tb * 512, tb * 512 + 512)
            eng = nc.gpsimd if tb % 2 == 0 else nc.sync
            eng.dma_start(out=xtq[:, :, sl], in_=xtq_r[:, :, sl])
        xt_r = xt_d.rearrange("(a p) t -> p a t", p=128)
        for tb in range(T // 512):
            sl = slice(tb * 512, tb * 512 + 512)
            eng = nc.sync if tb % 2 == 0 else nc.gpsimd
            eng.dma_start(out=xt[:, :, sl], in_=xt_r[:, :, sl])

        # precompute the 32 causal-mask tiles on the idle GPSIMD engine
        mk = [consts.tile([128, 512], bf16, name=f"mk_{i}") for i in range(32)]
        for i in range(32):
            nc.gpsimd.tensor_scalar(
                mk[i], ramp, thr[:, i:i + 1], None, op0=mybir.AluOpType.is_ge)

        # ---- single PSUM scope: pa 2 + pc/tr 2 + o_t 4 = 8 banks ----
        ot_all = consts.tile([128, QH // 128, HS], f32)
        with tc.tile_pool(name="psA", bufs=1, space="PSUM") as psA, \
             tc.tile_pool(name="psC", bufs=3, space="PSUM") as psC, \
             tc.tile_pool(name="psD", bufs=4, space="PSUM") as psD:
            for tb in range(QH // 512):   # A2: Q^T over the query rows
                sl = slice(tb * 512, tb * 512 + 512)
                pq = psA.tile([64, 512], f32, tag="pa", name=f"pq_{tb}")
                for ci in range(CCH):
                    nc.tensor.matmul(pq, wq[:, ci, :], xtq[:, ci, sl],
                                     start=(ci == 0), stop=(ci == CCH - 1))
                nc.vector.tensor_copy(qT[:, sl], pq)
            for tb in range(T // 512):    # A1: K^T | V^T over context
                sl = slice(tb * 512, tb * 512 + 512)
                pa = psA.tile([128, 512], f32, tag="pa", name=f"pa_{tb}")
                for ci in range(CCH):
                    nc.tensor.matmul(pa, wkv[:, ci, :], xt[:, ci, sl],
                                     start=(ci == 0), stop=(ci == CCH - 1))
                nc.vector.tensor_copy(kT[:, sl], pa[0:64, :])
                vts = epool.tile([64, 512], bf16, tag="vts", name=f"vts_{tb}")
                nc.vector.tensor_copy(vts, pa[64:128, :])
                for blk in range(4):
                    k = tb * 4 + blk
                    vtp = psA.tile([128, HS], bf16, tag="pa", name=f"vtp_{k}")
                    nc.tensor.transpose(
                        vtp, vts[:, blk * 128:blk * 128 + 128], id_bf)
                    nc.vector.tensor_copy(vp[:, k, 0:HS], vtp)

            # attention: slots round-robin by normalized progress so all
            # four chains stay live to the end (no serial tail)
            o_t = [psD.tile([65, 512], f32, tag="ot", name=f"o_t_{j}")
                   for j in range(NSLOT)]
            sched = []
            prog = [0] * NSLOT
            ends = [26, 28, 30, 32]   # staggered so finalizes overlap work
            for step in range(max(NCH)):
                for j in range(NSLOT - 1, -1, -1):
                    target = min(NCH[j], ((step + 1) * NCH[j] + ends[j] - 1)
                                 // ends[j])
                    while prog[j] < target:
                        sched.append((j, prog[j]))
                        prog[j] += 1
            for j, k in sched:
                ksl = slice(k * 128, k * 128 + 128)
                qsl = slice(j * 512, j * 512 + 512)
                pc = psC.tile([128, 512], f32, tag="pc", name=f"pc_{k}_{j}")
                nc.tensor.matmul(pc, kT[:, ksl], qT[:, qsl],
                                 start=True, stop=True)
                et = epool.tile([128, 512], bf16, tag="et",
                                name=f"et_{k}_{j}")
                nc.scalar.activation(et, pc, mybir.ActivationFunctionType.Exp)
                m = k - (NCH[j] - 8)
                if 0 <= m < 8:
                    nc.vector.tensor_mul(et, et, mk[8 * j + m])
                nc.tensor.matmul(o_t[j], vp[:, k, :], et,
                                 start=(k == 0), stop=(k == NCH[j] - 1))
                if k == NCH[j] - 1:   # finalize slot j now
                    ops = epool.tile([65, 512], f32, tag="ops",
                                     name=f"ops_{j}")
                    nc.vector.tensor_copy(ops, o_t[j])
                    for qs in range(4):
                        tp = psA.tile([128, HS + 1], f32, tag="pa",
                                      name=f"tp_{j}_{qs}")
                        nc.tensor.transpose(
                            tp, ops[:, qs * 128:qs * 128 + 128], id_f32)
                        rec = mpool.tile([128, 1], f32, tag="rec",
                                         name=f"rec_{j}_{qs}")
                        nc.vector.reciprocal(rec, tp[:, HS:HS + 1])
                        nc.vector.tensor_scalar_mul(
                            ot_all[:, 4 * j + qs, :], tp[:, 0:HS], rec)
        nc.sync.dma_start(
            out=out_d.rearrange("(q p) h -> p q h", p=128), in_=ot_all)

    nc.compile()
    return nc


def _prep_inputs(x, Wq, Wk, Wv):
    bf = ml_dtypes.bfloat16
    wkv = np.concatenate([Wk, Wv], axis=1).astype(bf)   # [C, 128]
    wq = (Wq * 0.125).astype(bf)
    ramp = np.broadcast_to(np.arange(512, dtype=np.float32), (128, 512)).copy()
    p = np.arange(128, dtype=np.float32)
    in_maps = []
    for core in range(8):
        b, h = core // 2, core % 2
        blocks = BLOCKS[h]
        xt = np.ascontiguousarray(x[b].T).astype(bf)
        xtq = np.concatenate(
            [x[b, g * 512:(g + 1) * 512] for g in blocks], axis=0
        ).T.astype(bf)
        thr = np.zeros((128, 32), np.float32)
        for j in range(NSLOT):
            for m in range(8):
                kk = NCH[j] - 8 + m
                thr[:, 8 * j + m] = 128 * kk + p - 512 * blocks[j]
        in_maps.append({
            "xt": np.ascontiguousarray(xt),
            "xtq": np.ascontiguousarray(xtq),
            "wkv": wkv, "wq": wq, "ramp": ramp, "thr": thr,
        })
    return in_maps


def kernel(x, Wq, Wk, Wv):
    from concourse.bass_utils import run_bass_kernel_spmd

    global _compiled
    if _compiled is None:
        _compiled = _build_program()
    nc = _compiled

    in_maps = _prep_inputs(
        np.asarray(x, np.float32), np.asarray(Wq, np.float32),
        np.asarray(Wk, np.float32), np.asarray(Wv, np.float32),
    )
    res = run_bass_kernel_spmd(nc, in_maps, list(range(8)))
    out = np.empty((B, T, HS), np.float32)
    for core in range(8):
        b, h = core // 2, core % 2
        o = res.results[core]["out"]
        for j, g in enumerate(BLOCKS[h]):
            out[b, g * 512:(g + 1) * 512] = o[j * 512:(j + 1) * 512]
    return out


if __name__ == "__main__":
    rng = np.random.default_rng(0)
    x = rng.standard_normal((B, T, C), dtype=np.float32)
    s = 1 / np.sqrt(C)
    Wq = rng.standard_normal((C, HS), dtype=np.float32) * s
    Wk = rng.standard_normal((C, HS), dtype=np.float32) * s
    Wv = rng.standard_normal((C, HS), dtype=np.float32) * s
    o = kernel(x=x, Wq=Wq, Wk=Wk, Wv=Wv)
    print(o.shape, o.dtype, np.abs(o).mean())

